# revision 6
# baseline (speedup 1.0000x reference)
"""Distributed Trainium2 (8 NeuronCores) attention kernel.

Reference computation (per batch b):
    q = rope(x @ wq.T), k = rope(x @ wk.T), v = x @ wv.T     (16 heads, hd=128)
    out = softmax(q k^T / sqrt(hd) + mask) v  @ wo.T

Sharding: core c handles batch b = c//4 and head-group g = c%4 (4 heads).
Per-core pipeline (all matmuls bf16 with fp32 PSUM accumulation):
  1. QT/KT = w.T-major projections straight into the transposed [j, s]
     layout the attention matmuls want; RoPE is applied with head-dims
     de-interleaved (host permutes wq/wk rows so rope pairs are
     (i, i+64) -> clean [64, 512] partition-block vector ops).
  2. Flash-style attention with transposed scores ST[k, q]:
     ST = KT_blk.T @ QT (contract over head dim), exp on ScalarE,
     PV as V_blk.T @ PT giving OT[j, q] directly (no transposes),
     softmax denominator via a ones-column matmul, normalization via
     reciprocal + ones-broadcast matmul + vector multiply.
  3. One 8-core AllToAll exchanges normalized OT so each core ends up
     with the full-head OT for its own 512-row sequence strip; rows of
     the other batch are neutralized by host-zeroed wo_big rows (keeps
     the SPMD graph core-independent).
  4. Local output projection -> out strip [512, 2048] fp32.

Host reassembles the 8 strips into the [2, 2048, 2048] output.
"""

import numpy as np
import ml_dtypes

import concourse.bass as bass
import concourse.bacc as bacc
import concourse.mybir as mybir
import concourse.tile as tile
from concourse.bass_utils import run_bass_kernel_spmd

BF16 = mybir.dt.bfloat16
F32 = mybir.dt.float32
NPBF16 = ml_dtypes.bfloat16

N_CORES = 8
B, S, D = 2, 2048, 2048
NH = 16            # total heads
HD = 128           # head dim
NHL = 4            # heads per core
JW = NHL * HD      # 512 local head width
NKT = D // 128     # 16 contraction tiles for projections
NQC = S // 512     # 4 sequence chunks of 512
NSB = S // 128     # 16 sequence blocks of 128
MASK_NEG = -60.0   # effective -inf for exp (scores are O(5))

_GRAPH_CACHE = {}


def build_graph(causal: bool):
    nc = bacc.Bacc("TRN2", target_bir_lowering=False, debug=False,
                   num_devices=N_CORES)

    # ---- per-core DRAM parameters -------------------------------------
    xT = nc.declare_dram_parameter("xT", [D, S], BF16, isOutput=False)
    wqT = nc.declare_dram_parameter("wqT", [D, JW], BF16, isOutput=False)
    wkT = nc.declare_dram_parameter("wkT", [D, JW], BF16, isOutput=False)
    wvT = nc.declare_dram_parameter("wvT", [D, JW], BF16, isOutput=False)
    wo_big = nc.declare_dram_parameter("wo_big", [8 * JW, D], BF16,
                                       isOutput=False)
    cosT = nc.declare_dram_parameter("cosT", [HD // 2, S], BF16, isOutput=False)
    sinT = nc.declare_dram_parameter("sinT", [HD // 2, S], BF16, isOutput=False)
    ones = nc.declare_dram_parameter("ones", [128, 128], BF16, isOutput=False)
    if causal:
        bnd = nc.declare_dram_parameter("bnd", [512, 512], F32, isOutput=False)
    else:
        maskT = nc.declare_dram_parameter("maskT", [S, S], BF16, isOutput=False)
    out = nc.declare_dram_parameter("out", [512, D], F32, isOutput=True)

    EXP = mybir.ActivationFunctionType.Exp

    with tile.TileContext(nc) as tc:
        with (
            tc.tile_pool(name="persist", bufs=1) as persist,
            tc.tile_pool(name="stream", bufs=3) as stream,
            tc.tile_pool(name="scratch", bufs=2) as scratch,
            tc.tile_pool(name="ps_mm", bufs=2, space="PSUM") as ps_mm,
            tc.tile_pool(name="ps_ot", bufs=2, space="PSUM") as ps_ot,
            tc.tile_pool(name="ps_sum", bufs=1, space="PSUM") as ps_sum,
            tc.tile_pool(name="ps_r", bufs=1, space="PSUM") as ps_r,
            tc.tile_pool(name="ps_o", bufs=2, space="PSUM") as ps_o,
            tc.tile_pool(name="dram", bufs=1, space="DRAM") as dram,
        ):
            ph1_cm = tc.tile_pool(name="ph1", bufs=1)
            ph1 = ph1_cm.__enter__()
            # ---- resident input tiles --------------------------------
            xt_sb = []
            for i in range(NKT):
                t = ph1.tile([128, S], BF16, tag=f"xt{i}", name=f"xt{i}")
                nc.sync.dma_start(t[:], xT[128 * i:128 * (i + 1), :])
                xt_sb.append(t)
            w_sb = {}
            for nm, h in (("q", wqT), ("k", wkT), ("v", wvT)):
                tiles = []
                for i in range(NKT):
                    t = ph1.tile([128, JW], BF16, tag=f"w{nm}{i}", name=f"w{nm}{i}")
                    nc.sync.dma_start(t[:], h[128 * i:128 * (i + 1), :])
                    tiles.append(t)
                w_sb[nm] = tiles
            cos_sb = persist.tile([HD // 2, S], BF16, tag="cos", name="cos")
            sin_sb = persist.tile([HD // 2, S], BF16, tag="sin", name="sin")
            nc.sync.dma_start(cos_sb[:], cosT[:, :])
            nc.sync.dma_start(sin_sb[:], sinT[:, :])
            ones_sb = persist.tile([128, 128], BF16, tag="ones", name="ones")
            nc.sync.dma_start(ones_sb[:], ones[:, :])
            if causal:
                bnd_sb = []
                for i in range(4):
                    t = persist.tile([128, 512], F32, tag=f"bnd{i}", name=f"bnd{i}")
                    nc.sync.dma_start(t[:], bnd[128 * i:128 * (i + 1), :])
                    bnd_sb.append(t)

            # attention working tensors (persist across phases)
            qt_sb = [persist.tile([128, S], BF16, tag=f"qt{h}", name=f"qt{h}")
                     for h in range(NHL)]
            kt_sb = [persist.tile([128, S], BF16, tag=f"kt{h}", name=f"kt{h}")
                     for h in range(NHL)]
            v_sb = [persist.tile([128, JW], BF16, tag=f"v{i}", name=f"v{i}")
                    for i in range(NSB)]

            cc_in = dram.tile([8 * JW, 512], BF16, tag="cc_in", name="cc_in")
            cc_out = dram.tile([8 * JW, 512], BF16, tag="cc_out", name="cc_out")

            # ---- phase 1: QKV projections + rope ---------------------
            def rope_into(dst, psum, qc):
                """dst[0:64] = A*cos - B*sin ; dst[64:128] = A*sin + B*cos."""
                cs = cos_sb[:, 512 * qc:512 * (qc + 1)]
                sn = sin_sb[:, 512 * qc:512 * (qc + 1)]
                A = psum[0:64, :]
                Bp = psum[64:128, :]
                t1 = scratch.tile([64, 512], F32, tag="r1", name="r1")
                t2 = scratch.tile([64, 512], F32, tag="r2", name="r2")
                nc.vector.tensor_mul(t1[:], A, cs)
                nc.vector.tensor_mul(t2[:], Bp, sn)
                nc.vector.tensor_sub(dst[0:64, 512 * qc:512 * (qc + 1)],
                                     t1[:], t2[:])
                t3 = scratch.tile([64, 512], F32, tag="r1", name="r1")
                t4 = scratch.tile([64, 512], F32, tag="r2", name="r2")
                nc.vector.tensor_mul(t3[:], A, sn)
                nc.vector.tensor_mul(t4[:], Bp, cs)
                nc.vector.tensor_add(dst[64:128, 512 * qc:512 * (qc + 1)],
                                     t3[:], t4[:])

            for qc in range(NQC):
                for h in range(NHL):
                    for nm, dsts in (("k", kt_sb), ("q", qt_sb)):
                        ps = ps_mm.tile([128, 512], F32, tag="mm", name="mm")
                        for dt in range(NKT):
                            nc.tensor.matmul(
                                ps[:],
                                w_sb[nm][dt][:, 128 * h:128 * (h + 1)],
                                xt_sb[dt][:, 512 * qc:512 * (qc + 1)],
                                start=(dt == 0), stop=(dt == NKT - 1),
                            )
                        rope_into(dsts[h], ps, qc)
            for sb_i in range(NSB):
                ps = ps_mm.tile([128, 512], F32, tag="mm", name="mm")
                for dt in range(NKT):
                    nc.tensor.matmul(
                        ps[:],
                        xt_sb[dt][:, 128 * sb_i:128 * (sb_i + 1)],
                        w_sb["v"][dt][:],
                        start=(dt == 0), stop=(dt == NKT - 1),
                    )
                nc.scalar.copy(v_sb[sb_i][:], ps[:])

            ph1_cm.__exit__(None, None, None)

            # ---- phase 2: attention ----------------------------------
            for qc in range(NQC):
                if not causal:
                    mt_sb = []
                    for kb in range(NSB):
                        t = stream.tile([128, 512], BF16, tag=f"mt{kb}", name=f"mt{kb}")
                        nc.sync.dma_start(
                            t[:], maskT[128 * kb:128 * (kb + 1),
                                        512 * qc:512 * (qc + 1)])
                        mt_sb.append(t)
                kbs = range(4 * qc + 4) if causal else range(NSB)
                for h in range(NHL):
                    ot_ps = ps_ot.tile([128, 512], F32, tag="ot", name="ot")
                    sum_ps = ps_sum.tile([1, 512], F32, tag="sum", name="sum")
                    last = kbs[-1]
                    for kb in kbs:
                        st = ps_mm.tile([128, 512], F32, tag="mm", name="mm")
                        nc.tensor.matmul(
                            st[:],
                            kt_sb[h][:, 128 * kb:128 * (kb + 1)],
                            qt_sb[h][:, 512 * qc:512 * (qc + 1)],
                            start=True, stop=True,
                        )
                        if causal:
                            if kb >= 4 * qc:
                                nc.vector.tensor_add(
                                    st[:], st[:], bnd_sb[kb - 4 * qc][:])
                        else:
                            nc.vector.tensor_add(st[:], st[:], mt_sb[kb][:])
                        pt = stream.tile([128, 512], BF16, tag="pt", name="pt")
                        nc.scalar.activation(pt[:], st[:], EXP)
                        nc.tensor.matmul(
                            ot_ps[:],
                            v_sb[kb][:, 128 * h:128 * (h + 1)],
                            pt[:],
                            start=(kb == 0), stop=(kb == last),
                        )
                        nc.tensor.matmul(
                            sum_ps[:],
                            ones_sb[:, 0:1],
                            pt[:],
                            start=(kb == 0), stop=(kb == last),
                        )
                    # normalize: r = 1/sum, R = broadcast(r), OTn = OT * R
                    r_sb = scratch.tile([1, 512], BF16, tag="r", name="r")
                    with nc.allow_low_precision(reason="softmax recip to bf16"):
                        nc.vector.reciprocal(r_sb[:], sum_ps[:])
                    rb_ps = ps_r.tile([128, 512], F32, tag="rb", name="rb")
                    nc.tensor.matmul(rb_ps[:], ones_sb[0:1, :], r_sb[:],
                                     start=True, stop=True)
                    rb_sb = scratch.tile([128, 512], F32, tag="rbs", name="rbs")
                    nc.scalar.copy(rb_sb[:], rb_ps[:])
                    otn = stream.tile([128, 512], BF16, tag="otn", name="otn")
                    nc.vector.tensor_mul(otn[:], ot_ps[:], rb_sb[:])
                    # ship to both twin shards (qc and qc+4)
                    for p in (qc, qc + 4):
                        nc.sync.dma_start(
                            cc_in[512 * p + 128 * h:512 * p + 128 * (h + 1), :],
                            otn[:])

            # ---- phase 3: AllToAll + output projection ---------------
            nc.gpsimd.collective_compute(
                "AllToAll",
                mybir.AluOpType.bypass,
                replica_groups=[list(range(N_CORES))],
                ins=[cc_in.opt()],
                outs=[cc_out.opt()],
            )
            ph3_cm = tc.tile_pool(name="ph3", bufs=1)
            ph3 = ph3_cm.__enter__()
            wopool_cm = tc.tile_pool(name="wopool", bufs=34)
            wopool = wopool_cm.__enter__()
            ot_rx = []
            for jt in range(32):
                t = ph3.tile([128, 512], BF16, tag=f"otr{jt}", name=f"otr{jt}")
                nc.sync.dma_start(t[:], cc_out[128 * jt:128 * (jt + 1), :])
                ot_rx.append(t)
            for mc in range(4):
                wo_tiles = []
                for jt in range(32):
                    t = wopool.tile([128, 512], BF16, tag="wo", name="wo")
                    nc.sync.dma_start(
                        t[:], wo_big[128 * jt:128 * (jt + 1),
                                     512 * mc:512 * (mc + 1)])
                    wo_tiles.append(t)
                for ss in range(4):
                    po = ps_o.tile([128, 512], F32, tag="po", name="po")
                    for jt in range(32):
                        nc.tensor.matmul(
                            po[:],
                            ot_rx[jt][:, 128 * ss:128 * (ss + 1)],
                            wo_tiles[jt][:],
                            start=(jt == 0), stop=(jt == 31),
                        )
                    os_sb = scratch.tile([128, 512], F32, tag="os", name="os")
                    nc.scalar.copy(os_sb[:], po[:])
                    nc.sync.dma_start(
                        out[128 * ss:128 * (ss + 1),
                            512 * mc:512 * (mc + 1)], os_sb[:])
            wopool_cm.__exit__(None, None, None)
            ph3_cm.__exit__(None, None, None)

    nc.compile()
    return nc


def _prep_inputs(x, freqs_cos, freqs_sin, mask, wq, wk, wv, wo, causal):
    perm = np.concatenate(
        [h * HD + np.r_[np.arange(0, HD, 2), np.arange(1, HD, 2)]
         for h in range(NHL)])
    cosT = np.ascontiguousarray(freqs_cos.T.astype(np.float32))
    sinT = np.ascontiguousarray(freqs_sin.T.astype(np.float32))
    ones = np.ones((128, 128), dtype=NPBF16)
    if causal:
        ki = np.arange(128)[:, None]
        qi = np.arange(512)[None, :]
        bnd = np.concatenate(
            [np.where(qi >= ki + 128 * i, 0.0, MASK_NEG)[None]
             for i in range(4)], axis=0).astype(np.float32).reshape(512, 512)
    else:
        maskT = np.ascontiguousarray(
            np.maximum(mask, MASK_NEG).T.astype(NPBF16))

    in_maps = []
    for c in range(N_CORES):
        b, g = c // 4, c % 4
        rows = slice(JW * g, JW * (g + 1))
        wq_c = wq[rows][perm] * (HD ** -0.5)
        wk_c = wk[rows][perm]
        wv_c = wv[rows]
        wo_big = np.zeros((8 * JW, D), dtype=np.float32)
        for p in range(8):
            if p // 4 == b:
                gp = p % 4
                wo_big[JW * p:JW * (p + 1)] = wo[:, JW * gp:JW * (gp + 1)].T
        m = {
            "xT": np.ascontiguousarray(x[b].T).astype(NPBF16),
            "wqT": np.ascontiguousarray(wq_c.T).astype(NPBF16),
            "wkT": np.ascontiguousarray(wk_c.T).astype(NPBF16),
            "wvT": np.ascontiguousarray(wv_c.T).astype(NPBF16),
            "wo_big": wo_big.astype(NPBF16),
            "cosT": cosT.astype(NPBF16),
            "sinT": sinT.astype(NPBF16),
            "ones": ones,
        }
        if causal:
            m["bnd"] = bnd
        else:
            m["maskT"] = maskT
        in_maps.append(m)
    return in_maps


def kernel(x, start_pos, freqs_cos, freqs_sin, mask, wq, wk, wv, wo):
    x = np.asarray(x, dtype=np.float32)
    mask = np.asarray(mask, dtype=np.float32)
    wq, wk, wv, wo = (np.asarray(w, dtype=np.float32) for w in (wq, wk, wv, wo))
    freqs_cos = np.asarray(freqs_cos, dtype=np.float32)
    freqs_sin = np.asarray(freqs_sin, dtype=np.float32)
    assert x.shape == (B, S, D) and mask.shape == (S, S)

    canonical = np.triu(np.full((S, S), float("-inf"), dtype=np.float32), k=1)
    causal = bool(np.array_equal(mask, canonical))

    if causal not in _GRAPH_CACHE:
        _GRAPH_CACHE[causal] = build_graph(causal)
    nc = _GRAPH_CACHE[causal]

    in_maps = _prep_inputs(x, freqs_cos, freqs_sin, mask, wq, wk, wv, wo,
                           causal)
    res = run_bass_kernel_spmd(nc, in_maps, core_ids=list(range(N_CORES)))
    out = np.empty((B, S, D), dtype=np.float32)
    for c in range(N_CORES):
        b, g = c // 4, c % 4
        out[b, JW * g:JW * (g + 1), :] = res.results[c]["out"]
    return out


# revision 9
# speedup vs baseline: 1.1196x; 1.1196x over previous
"""Distributed Trainium2 (8 NeuronCores) attention kernel.

Reference computation (per batch b):
    q = rope(x @ wq.T), k = rope(x @ wk.T), v = x @ wv.T     (16 heads, hd=128)
    out = softmax(q k^T / sqrt(hd) + mask) v  @ wo.T

Sharding: core c handles batch b = c//4 and head-group g = c%4 (4 heads).
Per-core pipeline (all matmuls bf16 with fp32 PSUM accumulation):
  1. QT/KT = w.T-major projections straight into the transposed [j, s]
     layout the attention matmuls want; RoPE is applied with head-dims
     de-interleaved (host permutes wq/wk rows so rope pairs are
     (i, i+64) -> clean [64, 512] partition-block vector ops).
  2. Flash-style attention with transposed scores ST[k, q]:
     ST = KT_blk.T @ QT (contract over head dim), exp on ScalarE,
     PV as V_blk.T @ PT giving OT[j, q] directly (no transposes),
     softmax denominator via a ones-column matmul, normalization via
     reciprocal + ones-broadcast matmul + vector multiply.
  3. One 8-core AllToAll exchanges normalized OT so each core ends up
     with the full-head OT for its own 512-row sequence strip; rows of
     the other batch are neutralized by host-zeroed wo_big rows (keeps
     the SPMD graph core-independent).
  4. Local output projection -> out strip [512, 2048] fp32.

Host reassembles the 8 strips into the [2, 2048, 2048] output.
"""

import numpy as np
import ml_dtypes

import concourse.bass as bass
import concourse.bacc as bacc
import concourse.mybir as mybir
import concourse.tile as tile
from concourse.bass_utils import run_bass_kernel_spmd

BF16 = mybir.dt.bfloat16
F32 = mybir.dt.float32
NPBF16 = ml_dtypes.bfloat16

N_CORES = 8
B, S, D = 2, 2048, 2048
NH = 16            # total heads
HD = 128           # head dim
NHL = 4            # heads per core
JW = NHL * HD      # 512 local head width
NKT = D // 128     # 16 contraction tiles for projections
NQC = S // 512     # 4 sequence chunks of 512
NSB = S // 128     # 16 sequence blocks of 128
MASK_NEG = -60.0   # effective -inf for exp (scores are O(5))

_GRAPH_CACHE = {}


def build_graph(causal: bool):
    nc = bacc.Bacc("TRN2", target_bir_lowering=False, debug=False,
                   num_devices=N_CORES)

    # ---- per-core DRAM parameters -------------------------------------
    xT = nc.declare_dram_parameter("xT", [D, S], BF16, isOutput=False)
    wqT = nc.declare_dram_parameter("wqT", [D, JW], BF16, isOutput=False)
    wkT = nc.declare_dram_parameter("wkT", [D, JW], BF16, isOutput=False)
    wvT = nc.declare_dram_parameter("wvT", [D, JW], BF16, isOutput=False)
    wo_big = nc.declare_dram_parameter("wo_big", [8 * JW, D], BF16,
                                       isOutput=False)
    cos2 = nc.declare_dram_parameter("cos2", [HD, S], BF16, isOutput=False)
    sgn2 = nc.declare_dram_parameter("sgn2", [HD, S], BF16, isOutput=False)
    ones = nc.declare_dram_parameter("ones", [128, 128], BF16, isOutput=False)
    eye = nc.declare_dram_parameter("eye", [128, 128], BF16, isOutput=False)
    if causal:
        bnd = nc.declare_dram_parameter("bnd", [512, 512], BF16, isOutput=False)
    else:
        maskT = nc.declare_dram_parameter("maskT", [S, S], BF16, isOutput=False)
    out = nc.declare_dram_parameter("out", [512, D], F32, isOutput=True)

    EXP = mybir.ActivationFunctionType.Exp

    with tile.TileContext(nc) as tc:
        with (
            tc.tile_pool(name="persist", bufs=1) as persist,
            tc.tile_pool(name="stream", bufs=3) as stream,
            tc.tile_pool(name="scratch", bufs=2) as scratch,
            tc.tile_pool(name="ps_mm", bufs=4, space="PSUM") as ps_mm,
            tc.tile_pool(name="ps_ot", bufs=2, space="PSUM") as ps_ot,
            tc.tile_pool(name="ps_sum", bufs=2, space="PSUM") as ps_sum,
            tc.tile_pool(name="dram", bufs=1, space="DRAM") as dram,
        ):
            ph1_cm = tc.tile_pool(name="ph1", bufs=1)
            ph1 = ph1_cm.__enter__()
            # ---- resident input tiles --------------------------------
            xt_sb = []
            for i in range(NKT):
                t = ph1.tile([128, S], BF16, tag=f"xt{i}", name=f"xt{i}")
                nc.sync.dma_start(t[:], xT[128 * i:128 * (i + 1), :])
                xt_sb.append(t)
            w_sb = {}
            for nm, h in (("q", wqT), ("k", wkT), ("v", wvT)):
                tiles = []
                for i in range(NKT):
                    t = ph1.tile([128, JW], BF16, tag=f"w{nm}{i}", name=f"w{nm}{i}")
                    nc.sync.dma_start(t[:], h[128 * i:128 * (i + 1), :])
                    tiles.append(t)
                w_sb[nm] = tiles
            cos_sb = persist.tile([HD, S], BF16, tag="cos", name="cos")
            sgn_sb = persist.tile([HD, S], BF16, tag="sin", name="sin")
            nc.sync.dma_start(cos_sb[:], cos2[:, :])
            nc.sync.dma_start(sgn_sb[:], sgn2[:, :])
            ones_sb = persist.tile([128, 128], BF16, tag="ones", name="ones")
            nc.sync.dma_start(ones_sb[:], ones[:, :])
            eye_sb = persist.tile([128, 128], BF16, tag="eye", name="eye")
            nc.sync.dma_start(eye_sb[:], eye[:, :])
            if causal:
                bnd_sb = []
                for i in range(4):
                    t = persist.tile([128, 512], BF16, tag=f"bnd{i}", name=f"bnd{i}")
                    nc.sync.dma_start(t[:], bnd[128 * i:128 * (i + 1), :])
                    bnd_sb.append(t)

            # attention working tensors (persist across phases)
            qt_sb = [persist.tile([128, S], BF16, tag=f"qt{h}", name=f"qt{h}")
                     for h in range(NHL)]
            kt_sb = [persist.tile([128, S], BF16, tag=f"kt{h}", name=f"kt{h}")
                     for h in range(NHL)]
            v_sb = [persist.tile([128, JW], BF16, tag=f"v{i}", name=f"v{i}")
                    for i in range(NSB)]

            cc_in = dram.tile([8 * JW, 512], BF16, tag="cc_in", name="cc_in")
            cc_out = dram.tile([8 * JW, 512], BF16, tag="cc_out", name="cc_out")

            # ---- phase 1: QKV projections + rope ---------------------
            def rope_into(dst, psum, qc):
                """Rope with de-interleaved head dims (pairs at i, i+64):
                dst = [A;B]*cos2 + [B;A]*sgn2  where sgn2 = [-sin; +sin].

                All DVE operands partition-aligned bf16 SBUF (2x mode)."""
                sl = slice(512 * qc, 512 * (qc + 1))
                stg = scratch.tile([128, 512], BF16, tag="stg", name="stg")
                nc.scalar.copy(stg[:], psum[:])
                sw = scratch.tile([128, 512], BF16, tag="sw", name="sw")
                nc.scalar.copy(sw[0:64, :], psum[64:128, :])
                nc.scalar.copy(sw[64:128, :], psum[0:64, :])
                u = scratch.tile([128, 512], BF16, tag="u", name="u")
                v = scratch.tile([128, 512], BF16, tag="v", name="v")
                nc.vector.tensor_mul(u[:], stg[:], cos_sb[:, sl])
                nc.vector.tensor_mul(v[:], sw[:], sgn_sb[:, sl])
                nc.vector.tensor_add(dst[:, sl], u[:], v[:])

            for qc in range(NQC):
                for h in range(NHL):
                    for nm, dsts in (("k", kt_sb), ("q", qt_sb)):
                        ps = ps_mm.tile([128, 512], F32, tag="mm", name="mm")
                        for dt in range(NKT):
                            nc.tensor.matmul(
                                ps[:],
                                w_sb[nm][dt][:, 128 * h:128 * (h + 1)],
                                xt_sb[dt][:, 512 * qc:512 * (qc + 1)],
                                start=(dt == 0), stop=(dt == NKT - 1),
                            )
                        rope_into(dsts[h], ps, qc)
            for sb_i in range(NSB):
                ps = ps_mm.tile([128, 512], F32, tag="mm", name="mm")
                for dt in range(NKT):
                    nc.tensor.matmul(
                        ps[:],
                        xt_sb[dt][:, 128 * sb_i:128 * (sb_i + 1)],
                        w_sb["v"][dt][:],
                        start=(dt == 0), stop=(dt == NKT - 1),
                    )
                nc.scalar.copy(v_sb[sb_i][:], ps[:])

            ph1_cm.__exit__(None, None, None)

            # ---- phase 2: attention ----------------------------------
            for qc in range(NQC):
                if not causal:
                    mt_sb = []
                    for kb in range(NSB):
                        t = stream.tile([128, 512], BF16, tag=f"mt{kb}", name=f"mt{kb}")
                        nc.sync.dma_start(
                            t[:], maskT[128 * kb:128 * (kb + 1),
                                        512 * qc:512 * (qc + 1)])
                        mt_sb.append(t)
                kbs = range(4 * qc + 4) if causal else range(NSB)
                for h in range(NHL):
                    ot_ps = ps_ot.tile([128, 512], F32, tag="ot", name="ot")
                    sum_ps = ps_sum.tile([1, 512], F32, tag="sum", name="sum")
                    last = kbs[-1]
                    for kb in kbs:
                        st = ps_mm.tile([128, 512], F32, tag="mm", name="mm")
                        has_mask = (kb >= 4 * qc) if causal else True
                        nc.tensor.matmul(
                            st[:],
                            kt_sb[h][:, 128 * kb:128 * (kb + 1)],
                            qt_sb[h][:, 512 * qc:512 * (qc + 1)],
                            start=True, stop=not has_mask,
                        )
                        if has_mask:
                            m_rhs = (bnd_sb[kb - 4 * qc] if causal
                                     else mt_sb[kb])
                            nc.tensor.matmul(st[:], eye_sb[:], m_rhs[:],
                                             start=False, stop=True)
                        pt = stream.tile([128, 512], BF16, tag="pt", name="pt")
                        nc.scalar.activation(pt[:], st[:], EXP)
                        nc.tensor.matmul(
                            ot_ps[:],
                            v_sb[kb][:, 128 * h:128 * (h + 1)],
                            pt[:],
                            start=(kb == 0), stop=(kb == last),
                        )
                        nc.tensor.matmul(
                            sum_ps[:],
                            ones_sb[:, 0:1],
                            pt[:],
                            start=(kb == 0), stop=(kb == last),
                        )
                    # normalize: broadcast sums, reciprocal, OTn = OT * R
                    sum_sb = scratch.tile([1, 512], BF16, tag="sumsb",
                                          name="sumsb")
                    with nc.allow_low_precision(reason="softmax sums to bf16"):
                        nc.scalar.copy(sum_sb[:], sum_ps[:])
                    rb_ps = ps_mm.tile([128, 512], F32, tag="mm", name="mm")
                    nc.tensor.matmul(rb_ps[:], ones_sb[0:1, :], sum_sb[:],
                                     start=True, stop=True)
                    rb_sb = scratch.tile([128, 512], F32, tag="rbs", name="rbs")
                    nc.vector.reciprocal(rb_sb[:], rb_ps[:])
                    otn = stream.tile([128, 512], BF16, tag="otn", name="otn")
                    nc.vector.tensor_mul(otn[:], ot_ps[:], rb_sb[:])
                    # ship to both twin shards (qc and qc+4)
                    for p in (qc, qc + 4):
                        nc.sync.dma_start(
                            cc_in[512 * p + 128 * h:512 * p + 128 * (h + 1), :],
                            otn[:])

            # ---- phase 3: AllToAll + output projection ---------------
            wopool_cm = tc.tile_pool(name="wopool", bufs=72)
            wopool = wopool_cm.__enter__()
            wo_tiles = {}
            for mc in range(4):
                for jt in range(32):
                    t = wopool.tile([128, 512], BF16, tag="wo", name="wo")
                    nc.sync.dma_start(
                        t[:], wo_big[128 * jt:128 * (jt + 1),
                                     512 * mc:512 * (mc + 1)])
                    wo_tiles[(mc, jt)] = t
            nc.gpsimd.collective_compute(
                "AllToAll",
                mybir.AluOpType.bypass,
                replica_groups=[list(range(N_CORES))],
                ins=[cc_in.opt()],
                outs=[cc_out.opt()],
            )
            ph3_cm = tc.tile_pool(name="ph3", bufs=1)
            ph3 = ph3_cm.__enter__()
            ot_rx = []
            for jt in range(32):
                t = ph3.tile([128, 512], BF16, tag=f"otr{jt}", name=f"otr{jt}")
                nc.sync.dma_start(t[:], cc_out[128 * jt:128 * (jt + 1), :])
                ot_rx.append(t)
            for mc in range(4):
                for ss in range(4):
                    po = ps_mm.tile([128, 512], F32, tag="mm", name="mm")
                    for jt in range(32):
                        nc.tensor.matmul(
                            po[:],
                            ot_rx[jt][:, 128 * ss:128 * (ss + 1)],
                            wo_tiles[(mc, jt)][:],
                            start=(jt == 0), stop=(jt == 31),
                        )
                    os_sb = scratch.tile([128, 512], F32, tag="os", name="os")
                    nc.scalar.copy(os_sb[:], po[:])
                    nc.sync.dma_start(
                        out[128 * ss:128 * (ss + 1),
                            512 * mc:512 * (mc + 1)], os_sb[:])
            ph3_cm.__exit__(None, None, None)
            wopool_cm.__exit__(None, None, None)

    nc.compile()
    return nc


def _prep_inputs(x, freqs_cos, freqs_sin, mask, wq, wk, wv, wo, causal):
    perm = np.concatenate(
        [h * HD + np.r_[np.arange(0, HD, 2), np.arange(1, HD, 2)]
         for h in range(NHL)])
    cosT = np.ascontiguousarray(freqs_cos.T.astype(np.float32))  # [64, S]
    sinT = np.ascontiguousarray(freqs_sin.T.astype(np.float32))
    cos2 = np.concatenate([cosT, cosT], axis=0)           # [128, S]
    sgn2 = np.concatenate([-sinT, sinT], axis=0)          # [128, S]
    ones = np.ones((128, 128), dtype=NPBF16)
    eye = np.eye(128, dtype=np.float32).astype(NPBF16)
    if causal:
        ki = np.arange(128)[:, None]
        qi = np.arange(512)[None, :]
        bnd = np.concatenate(
            [np.where(qi >= ki + 128 * i, 0.0, MASK_NEG)[None]
             for i in range(4)], axis=0).astype(np.float32).reshape(512, 512).astype(NPBF16)
    else:
        maskT = np.ascontiguousarray(
            np.maximum(mask, MASK_NEG).T.astype(NPBF16))

    in_maps = []
    for c in range(N_CORES):
        b, g = c // 4, c % 4
        rows = slice(JW * g, JW * (g + 1))
        wq_c = wq[rows][perm] * (HD ** -0.5)
        wk_c = wk[rows][perm]
        wv_c = wv[rows]
        wo_big = np.zeros((8 * JW, D), dtype=np.float32)
        for p in range(8):
            if p // 4 == b:
                gp = p % 4
                wo_big[JW * p:JW * (p + 1)] = wo[:, JW * gp:JW * (gp + 1)].T
        m = {
            "xT": np.ascontiguousarray(x[b].T).astype(NPBF16),
            "wqT": np.ascontiguousarray(wq_c.T).astype(NPBF16),
            "wkT": np.ascontiguousarray(wk_c.T).astype(NPBF16),
            "wvT": np.ascontiguousarray(wv_c.T).astype(NPBF16),
            "wo_big": wo_big.astype(NPBF16),
            "cos2": cos2.astype(NPBF16),
            "sgn2": sgn2.astype(NPBF16),
            "ones": ones,
            "eye": eye,
        }
        if causal:
            m["bnd"] = bnd
        else:
            m["maskT"] = maskT
        in_maps.append(m)
    return in_maps


def kernel(x, start_pos, freqs_cos, freqs_sin, mask, wq, wk, wv, wo):
    x = np.asarray(x, dtype=np.float32)
    mask = np.asarray(mask, dtype=np.float32)
    wq, wk, wv, wo = (np.asarray(w, dtype=np.float32) for w in (wq, wk, wv, wo))
    freqs_cos = np.asarray(freqs_cos, dtype=np.float32)
    freqs_sin = np.asarray(freqs_sin, dtype=np.float32)
    assert x.shape == (B, S, D) and mask.shape == (S, S)

    canonical = np.triu(np.full((S, S), float("-inf"), dtype=np.float32), k=1)
    causal = bool(np.array_equal(mask, canonical))

    if causal not in _GRAPH_CACHE:
        _GRAPH_CACHE[causal] = build_graph(causal)
    nc = _GRAPH_CACHE[causal]

    in_maps = _prep_inputs(x, freqs_cos, freqs_sin, mask, wq, wk, wv, wo,
                           causal)
    res = run_bass_kernel_spmd(nc, in_maps, core_ids=list(range(N_CORES)))
    out = np.empty((B, S, D), dtype=np.float32)
    for c in range(N_CORES):
        b, g = c // 4, c % 4
        out[b, JW * g:JW * (g + 1), :] = res.results[c]["out"]
    return out


# revision 11
# speedup vs baseline: 1.2427x; 1.1100x over previous
"""Distributed Trainium2 (8 NeuronCores) attention kernel.

Reference computation (per batch b):
    q = rope(x @ wq.T), k = rope(x @ wk.T), v = x @ wv.T     (16 heads, hd=128)
    out = softmax(q k^T / sqrt(hd) + mask) v  @ wo.T

Sharding: core c handles batch b = c//4 and head-group g = c%4 (4 heads).
Per-core pipeline (all matmuls bf16 with fp32 PSUM accumulation):
  1. QT/KT = w.T-major projections straight into the transposed [j, s]
     layout the attention matmuls want; RoPE is applied with head-dims
     de-interleaved (host permutes wq/wk rows so rope pairs are
     (i, i+64) -> clean [64, 512] partition-block vector ops).
  2. Flash-style attention with transposed scores ST[k, q]:
     ST = KT_blk.T @ QT (contract over head dim), exp on ScalarE,
     PV as V_blk.T @ PT giving OT[j, q] directly (no transposes),
     softmax denominator via a ones-column matmul, normalization via
     reciprocal + ones-broadcast matmul + vector multiply.
  3. One 8-core AllToAll exchanges normalized OT so each core ends up
     with the full-head OT for its own 512-row sequence strip; rows of
     the other batch are neutralized by host-zeroed wo_big rows (keeps
     the SPMD graph core-independent).
  4. Local output projection -> out strip [512, 2048] fp32.

Host reassembles the 8 strips into the [2, 2048, 2048] output.
"""

import numpy as np
import ml_dtypes

import concourse.bass as bass
import concourse.bacc as bacc
import concourse.mybir as mybir
import concourse.tile as tile
from concourse.bass_utils import run_bass_kernel_spmd

BF16 = mybir.dt.bfloat16
F32 = mybir.dt.float32
NPBF16 = ml_dtypes.bfloat16

N_CORES = 8
B, S, D = 2, 2048, 2048
NH = 16            # total heads
HD = 128           # head dim
NHL = 4            # heads per core
JW = NHL * HD      # 512 local head width
NKT = D // 128     # 16 contraction tiles for projections
NQC = S // 512     # 4 sequence chunks of 512
NSB = S // 128     # 16 sequence blocks of 128
MASK_NEG = -60.0   # effective -inf for exp (scores are O(5))

_GRAPH_CACHE = {}


def build_graph(causal: bool):
    nc = bacc.Bacc("TRN2", target_bir_lowering=False, debug=False,
                   num_devices=N_CORES)

    # ---- per-core DRAM parameters -------------------------------------
    xT = nc.declare_dram_parameter("xT", [D, S], BF16, isOutput=False)
    wqT = nc.declare_dram_parameter("wqT", [D, JW], BF16, isOutput=False)
    wkT = nc.declare_dram_parameter("wkT", [D, JW], BF16, isOutput=False)
    wvT = nc.declare_dram_parameter("wvT", [D, JW], BF16, isOutput=False)
    wo_big = nc.declare_dram_parameter("wo_big", [8 * JW, D], BF16,
                                       isOutput=False)
    cos2 = nc.declare_dram_parameter("cos2", [HD, S], BF16, isOutput=False)
    sgn2 = nc.declare_dram_parameter("sgn2", [HD, S], BF16, isOutput=False)
    ones = nc.declare_dram_parameter("ones", [128, 128], BF16, isOutput=False)
    eye = nc.declare_dram_parameter("eye", [128, 128], BF16, isOutput=False)
    if causal:
        bnd = nc.declare_dram_parameter("bnd", [512, 512], BF16, isOutput=False)
    else:
        maskT = nc.declare_dram_parameter("maskT", [S, S], BF16, isOutput=False)
    out = nc.declare_dram_parameter("out", [512, D], F32, isOutput=True)

    EXP = mybir.ActivationFunctionType.Exp

    with tile.TileContext(nc) as tc:
        with (
            tc.tile_pool(name="persist", bufs=1) as persist,
            tc.tile_pool(name="stream", bufs=3) as stream,
            tc.tile_pool(name="scratch", bufs=2) as scratch,
            tc.tile_pool(name="ps_mm", bufs=4, space="PSUM") as ps_mm,
            tc.tile_pool(name="ps_ot", bufs=2, space="PSUM") as ps_ot,
            tc.tile_pool(name="ps_sum", bufs=2, space="PSUM") as ps_sum,
            tc.tile_pool(name="dram", bufs=1, space="DRAM") as dram,
        ):
            ph1_cm = tc.tile_pool(name="ph1", bufs=1)
            ph1 = ph1_cm.__enter__()
            # ---- resident input tiles --------------------------------
            xt_sb = []
            for i in range(NKT):
                t = ph1.tile([128, S], BF16, tag=f"xt{i}", name=f"xt{i}")
                nc.sync.dma_start(t[:], xT[128 * i:128 * (i + 1), :])
                xt_sb.append(t)
            w_sb = {}
            for nm, h in (("q", wqT), ("k", wkT), ("v", wvT)):
                tiles = []
                for i in range(NKT):
                    t = ph1.tile([128, JW], BF16, tag=f"w{nm}{i}", name=f"w{nm}{i}")
                    nc.sync.dma_start(t[:], h[128 * i:128 * (i + 1), :])
                    tiles.append(t)
                w_sb[nm] = tiles
            cos_sb = persist.tile([HD, S], BF16, tag="cos", name="cos")
            sgn_sb = persist.tile([HD, S], BF16, tag="sin", name="sin")
            nc.sync.dma_start(cos_sb[:], cos2[:, :])
            nc.sync.dma_start(sgn_sb[:], sgn2[:, :])
            ones_sb = persist.tile([128, 128], BF16, tag="ones", name="ones")
            nc.sync.dma_start(ones_sb[:], ones[:, :])
            eye_sb = persist.tile([128, 128], BF16, tag="eye", name="eye")
            nc.sync.dma_start(eye_sb[:], eye[:, :])
            if causal:
                bnd_sb = []
                for i in range(4):
                    t = persist.tile([128, 512], BF16, tag=f"bnd{i}", name=f"bnd{i}")
                    nc.sync.dma_start(t[:], bnd[128 * i:128 * (i + 1), :])
                    bnd_sb.append(t)

            # attention working tensors (persist across phases)
            qt_sb = [persist.tile([128, S], BF16, tag=f"qt{h}", name=f"qt{h}")
                     for h in range(NHL)]
            kt_sb = [persist.tile([128, S], BF16, tag=f"kt{h}", name=f"kt{h}")
                     for h in range(NHL)]
            v_sb = [persist.tile([128, JW], BF16, tag=f"v{i}", name=f"v{i}")
                    for i in range(NSB)]

            cc_in = dram.tile([8 * JW, 512], BF16, tag="cc_in", name="cc_in")
            cc_out = dram.tile([8 * JW, 512], BF16, tag="cc_out", name="cc_out")

            # ---- phase 1: QKV projections + rope ---------------------
            def rope_into(dst, psum, qc):
                """Rope with de-interleaved head dims (pairs at i, i+64):
                dst = [A;B]*cos2 + [B;A]*sgn2  where sgn2 = [-sin; +sin].

                All DVE operands partition-aligned bf16 SBUF (2x mode)."""
                sl = slice(512 * qc, 512 * (qc + 1))
                stg = scratch.tile([128, 512], BF16, tag="stg", name="stg")
                nc.scalar.copy(stg[:], psum[:])
                sw = scratch.tile([128, 512], BF16, tag="sw", name="sw")
                nc.vector.tensor_copy(sw[0:64, :], stg[64:128, :])
                nc.vector.tensor_copy(sw[64:128, :], stg[0:64, :])
                u = scratch.tile([128, 512], BF16, tag="u", name="u")
                v = scratch.tile([128, 512], BF16, tag="v", name="v")
                nc.vector.tensor_mul(u[:], stg[:], cos_sb[:, sl])
                nc.vector.tensor_mul(v[:], sw[:], sgn_sb[:, sl])
                nc.vector.tensor_add(dst[:, sl], u[:], v[:])

            for qc in range(NQC):
                for h in range(NHL):
                    for nm, dsts in (("k", kt_sb), ("q", qt_sb)):
                        ps = ps_mm.tile([128, 512], F32, tag="mm", name="mm")
                        for dt in range(NKT):
                            nc.tensor.matmul(
                                ps[:],
                                w_sb[nm][dt][:, 128 * h:128 * (h + 1)],
                                xt_sb[dt][:, 512 * qc:512 * (qc + 1)],
                                start=(dt == 0), stop=(dt == NKT - 1),
                            )
                        rope_into(dsts[h], ps, qc)
            for sb_i in range(NSB):
                ps = ps_mm.tile([128, 512], F32, tag="mm", name="mm")
                for dt in range(NKT):
                    nc.tensor.matmul(
                        ps[:],
                        xt_sb[dt][:, 128 * sb_i:128 * (sb_i + 1)],
                        w_sb["v"][dt][:],
                        start=(dt == 0), stop=(dt == NKT - 1),
                    )
                nc.scalar.copy(v_sb[sb_i][:], ps[:])

            ph1_cm.__exit__(None, None, None)

            # ---- phase 2: attention ----------------------------------
            for qc in range(NQC):
                if not causal:
                    mt_sb = []
                    for kb in range(NSB):
                        t = stream.tile([128, 512], BF16, tag=f"mt{kb}", name=f"mt{kb}")
                        nc.sync.dma_start(
                            t[:], maskT[128 * kb:128 * (kb + 1),
                                        512 * qc:512 * (qc + 1)])
                        mt_sb.append(t)
                kbs = range(4 * qc + 4) if causal else range(NSB)
                for h in range(NHL):
                    ot_ps = ps_ot.tile([128, 512], F32, tag="ot", name="ot")
                    sum_ps = ps_sum.tile([1, 512], F32, tag="sum", name="sum")
                    last = kbs[-1]
                    for kb in kbs:
                        st = ps_mm.tile([128, 512], F32, tag="mm", name="mm")
                        has_mask = (kb >= 4 * qc) if causal else True
                        nc.tensor.matmul(
                            st[:],
                            kt_sb[h][:, 128 * kb:128 * (kb + 1)],
                            qt_sb[h][:, 512 * qc:512 * (qc + 1)],
                            start=True, stop=not has_mask,
                        )
                        if has_mask:
                            m_rhs = (bnd_sb[kb - 4 * qc] if causal
                                     else mt_sb[kb])
                            nc.tensor.matmul(st[:], eye_sb[:], m_rhs[:],
                                             start=False, stop=True)
                        pt = stream.tile([128, 512], BF16, tag="pt", name="pt")
                        nc.scalar.activation(pt[:], st[:], EXP)
                        nc.tensor.matmul(
                            ot_ps[:],
                            v_sb[kb][:, 128 * h:128 * (h + 1)],
                            pt[:],
                            start=(kb == 0), stop=(kb == last),
                        )
                        nc.tensor.matmul(
                            sum_ps[:],
                            ones_sb[:, 0:1],
                            pt[:],
                            start=(kb == 0), stop=(kb == last),
                        )
                    # normalize: r = approx 1/sums, partition-broadcast on
                    # gpsimd, OTn = OT * R
                    r_sb = scratch.tile([1, 512], F32, tag="rsb", name="rsb")
                    nc.vector.reciprocal_approx_fast(r_sb[:], sum_ps[:])
                    rb_sb = scratch.tile([128, 512], F32, tag="rbs", name="rbs")
                    nc.gpsimd.partition_broadcast(rb_sb[:], r_sb[:])
                    otn = stream.tile([128, 512], BF16, tag="otn", name="otn")
                    nc.vector.tensor_mul(otn[:], ot_ps[:], rb_sb[:])
                    # ship to both twin shards (qc and qc+4)
                    for p in (qc, qc + 4):
                        nc.sync.dma_start(
                            cc_in[512 * p + 128 * h:512 * p + 128 * (h + 1), :],
                            otn[:])

            # ---- phase 3: AllToAll + output projection ---------------
            wopool_cm = tc.tile_pool(name="wopool", bufs=72)
            wopool = wopool_cm.__enter__()
            wo_tiles = {}
            for mc in range(4):
                for jt in range(32):
                    t = wopool.tile([128, 512], BF16, tag="wo", name="wo")
                    nc.sync.dma_start(
                        t[:], wo_big[128 * jt:128 * (jt + 1),
                                     512 * mc:512 * (mc + 1)])
                    wo_tiles[(mc, jt)] = t
            nc.gpsimd.collective_compute(
                "AllToAll",
                mybir.AluOpType.bypass,
                replica_groups=[list(range(N_CORES))],
                ins=[cc_in.opt()],
                outs=[cc_out.opt()],
            )
            ph3_cm = tc.tile_pool(name="ph3", bufs=1)
            ph3 = ph3_cm.__enter__()
            ot_rx = []
            for jt in range(32):
                t = ph3.tile([128, 512], BF16, tag=f"otr{jt}", name=f"otr{jt}")
                nc.sync.dma_start(t[:], cc_out[128 * jt:128 * (jt + 1), :])
                ot_rx.append(t)
            for mc in range(4):
                for ss in range(4):
                    po = ps_mm.tile([128, 512], F32, tag="mm", name="mm")
                    for jt in range(32):
                        nc.tensor.matmul(
                            po[:],
                            ot_rx[jt][:, 128 * ss:128 * (ss + 1)],
                            wo_tiles[(mc, jt)][:],
                            start=(jt == 0), stop=(jt == 31),
                        )
                    os_sb = scratch.tile([128, 512], F32, tag="os", name="os")
                    nc.scalar.copy(os_sb[:], po[:])
                    nc.sync.dma_start(
                        out[128 * ss:128 * (ss + 1),
                            512 * mc:512 * (mc + 1)], os_sb[:])
            ph3_cm.__exit__(None, None, None)
            wopool_cm.__exit__(None, None, None)

    nc.compile()
    return nc


def _prep_inputs(x, freqs_cos, freqs_sin, mask, wq, wk, wv, wo, causal):
    perm = np.concatenate(
        [h * HD + np.r_[np.arange(0, HD, 2), np.arange(1, HD, 2)]
         for h in range(NHL)])
    cosT = np.ascontiguousarray(freqs_cos.T.astype(np.float32))  # [64, S]
    sinT = np.ascontiguousarray(freqs_sin.T.astype(np.float32))
    cos2 = np.concatenate([cosT, cosT], axis=0)           # [128, S]
    sgn2 = np.concatenate([-sinT, sinT], axis=0)          # [128, S]
    ones = np.ones((128, 128), dtype=NPBF16)
    eye = np.eye(128, dtype=np.float32).astype(NPBF16)
    if causal:
        ki = np.arange(128)[:, None]
        qi = np.arange(512)[None, :]
        bnd = np.concatenate(
            [np.where(qi >= ki + 128 * i, 0.0, MASK_NEG)[None]
             for i in range(4)], axis=0).astype(np.float32).reshape(512, 512).astype(NPBF16)
    else:
        maskT = np.ascontiguousarray(
            np.maximum(mask, MASK_NEG).T.astype(NPBF16))

    in_maps = []
    for c in range(N_CORES):
        b, g = c // 4, c % 4
        rows = slice(JW * g, JW * (g + 1))
        wq_c = wq[rows][perm] * (HD ** -0.5)
        wk_c = wk[rows][perm]
        wv_c = wv[rows]
        wo_big = np.zeros((8 * JW, D), dtype=np.float32)
        for p in range(8):
            if p // 4 == b:
                gp = p % 4
                wo_big[JW * p:JW * (p + 1)] = wo[:, JW * gp:JW * (gp + 1)].T
        m = {
            "xT": np.ascontiguousarray(x[b].T).astype(NPBF16),
            "wqT": np.ascontiguousarray(wq_c.T).astype(NPBF16),
            "wkT": np.ascontiguousarray(wk_c.T).astype(NPBF16),
            "wvT": np.ascontiguousarray(wv_c.T).astype(NPBF16),
            "wo_big": wo_big.astype(NPBF16),
            "cos2": cos2.astype(NPBF16),
            "sgn2": sgn2.astype(NPBF16),
            "ones": ones,
            "eye": eye,
        }
        if causal:
            m["bnd"] = bnd
        else:
            m["maskT"] = maskT
        in_maps.append(m)
    return in_maps


def kernel(x, start_pos, freqs_cos, freqs_sin, mask, wq, wk, wv, wo):
    x = np.asarray(x, dtype=np.float32)
    mask = np.asarray(mask, dtype=np.float32)
    wq, wk, wv, wo = (np.asarray(w, dtype=np.float32) for w in (wq, wk, wv, wo))
    freqs_cos = np.asarray(freqs_cos, dtype=np.float32)
    freqs_sin = np.asarray(freqs_sin, dtype=np.float32)
    assert x.shape == (B, S, D) and mask.shape == (S, S)

    canonical = np.triu(np.full((S, S), float("-inf"), dtype=np.float32), k=1)
    causal = bool(np.array_equal(mask, canonical))

    if causal not in _GRAPH_CACHE:
        _GRAPH_CACHE[causal] = build_graph(causal)
    nc = _GRAPH_CACHE[causal]

    in_maps = _prep_inputs(x, freqs_cos, freqs_sin, mask, wq, wk, wv, wo,
                           causal)
    res = run_bass_kernel_spmd(nc, in_maps, core_ids=list(range(N_CORES)))
    out = np.empty((B, S, D), dtype=np.float32)
    for c in range(N_CORES):
        b, g = c // 4, c % 4
        out[b, JW * g:JW * (g + 1), :] = res.results[c]["out"]
    return out
